# revision 18
# baseline (speedup 1.0000x reference)
"""Trainium2 Bass kernel for nn_Covid19InfectionsPredictModel.

Model: rate = relu(inputs @ a)  [T];  a strictly-sequential 20-tap linear
recurrence s_t = rate_t * dot(s_{t-20..t-1}, rev_head) seeded by a unit
impulse, and the observation FIR out_t = dot(s_{t-20..t-1}, diff).

Key structural fact (pure IEEE-754 float32, no approximation): the weight
rev_head[0] is exactly 0 and diff[j] > 0 for all j. The recurrence explodes
(growth ~14x/step for the given parameter scale), hits +inf, and then
0*inf => NaN poisons the window. Once the 20-value state window is ALL-NaN
at some step t*, every later s_t and out_t is NaN *regardless of rate*
(NaN*w + ... = NaN, and rate*NaN = NaN even for rate == 0). So the exact
full output is determined by the first ~60 steps plus a runtime-verified
all-NaN certificate.

Strategy:
  1. One small NEFF computes the first K=96 steps exactly on-device:
     rate via TensorE matmuls (the host passes the 96-row input slice
     pre-transposed - a pure layout change), per-step pre-scaled weights
     W[t] = rate_t * rev_head as a TensorE outer product flattened
     through a DRAM bounce, the sequential recurrence as ONE fused DVE
     scalar_tensor_tensor (accum_out) per step, the output FIR as a
     20-tap scalar_tensor_tensor chain, plus the NaN tail out[96:] as a
     single DRAM->DRAM broadcast DMA and the final 20-value state window.
  2. Host checks the returned state window. All-NaN (the certified,
     overwhelmingly common case) => done. Otherwise a fallback chunk
     NEFF (1024 steps per launch, same math) is compiled lazily and
     looped over the remaining sequence, so the kernel is exactly
     correct for ANY inputs, just slower in the never-taken branch.

The recurrence is strictly sequential and tiny per step; there is no
batch/scenario dimension in this problem instance, so the 8 cores run the
program SPMD-replicated (core 0's output is gathered) - intra-sequence
sharding has nothing to win (see sharding hint). The sequential step is
latency-bound at one small DVE op (~165ns); step-pairing rewrites were
rejected because they provably shift the inf->NaN onset by one position.
"""

from contextlib import ExitStack

import numpy as np

T_FULL = 65536
NW = 64
L = 21
WIN = 20  # recurrence window (L-1 taps)
K_HAPPY = 96
K_CHUNK = 1024
NAN_COLS = 2045  # (T_FULL - K_HAPPY) == 32 * 2045

_cache = {}


def _weights_from_h(h):
    """rev_head (wvec) and diff (dvec) exactly as the reference computes them."""
    h0 = np.float32(np.asarray(h).reshape(-1)[0])
    t = np.arange(L, dtype=np.float32)
    surv = np.exp(-t / h0).astype(np.float32)
    surv = ((surv - surv[-1]) / (np.float32(1.0) - surv[-1])).astype(np.float32)
    rev = surv[::-1].copy()  # reverse_surv, rev[20] == 1, rev[0] == 0
    wvec = rev[:WIN].copy()  # rev_head, wvec[0] == 0 exactly
    dvec = (rev[1:] - rev[:-1]).copy()  # diff, all > 0
    return wvec, dvec


def _build(K, full_out):
    """Emit + compile the K-step recurrence NEFF. Returns the Bacc object."""
    import concourse.bacc as bacc
    import concourse.bass as bass
    import concourse.tile as tile
    from concourse import mybir

    f32 = mybir.dt.float32
    mult = mybir.AluOpType.mult
    add = mybir.AluOpType.add

    nc = bacc.Bacc("TRN2", num_devices=1, debug=False)
    xT = nc.dram_tensor("xT", [NW, K], f32, kind="ExternalInput").ap()
    a = nc.dram_tensor("a", [NW], f32, kind="ExternalInput").ap()
    wv = nc.dram_tensor("wv", [WIN], f32, kind="ExternalInput").ap()
    dv = nc.dram_tensor("dv", [WIN], f32, kind="ExternalInput").ap()
    st_in = nc.dram_tensor("st_in", [WIN], f32, kind="ExternalInput").ap()
    if full_out:
        nansrc = nc.dram_tensor("nansrc", [NAN_COLS], f32, kind="ExternalInput").ap()
        out = nc.dram_tensor("out", [T_FULL], f32, kind="ExternalOutput").ap()
    else:
        out = nc.dram_tensor("out", [K], f32, kind="ExternalOutput").ap()
    st_out = nc.dram_tensor("st_out", [WIN], f32, kind="ExternalOutput").ap()

    with tile.TileContext(nc) as tc, ExitStack() as ctx:
        pool = ctx.enter_context(tc.tile_pool(name="p", bufs=1))
        psp = ctx.enter_context(tc.tile_pool(name="ps", bufs=1, space="PSUM"))

        xT_sb = pool.tile([NW, K], f32)
        nc.sync.dma_start(out=xT_sb, in_=xT)
        a_sb = pool.tile([NW, 1], f32)
        nc.sync.dma_start(out=a_sb, in_=a.rearrange("(k o) -> k o", o=1))
        w_row = pool.tile([1, WIN], f32)
        nc.sync.dma_start(out=w_row, in_=wv.rearrange("(o k) -> o k", o=1))
        d_row = pool.tile([1, WIN], f32)
        nc.sync.dma_start(out=d_row, in_=dv.rearrange("(o k) -> o k", o=1))
        s_buf = pool.tile([1, K + WIN], f32)
        nc.sync.dma_start(out=s_buf[:, 0:WIN], in_=st_in.rearrange("(o k) -> o k", o=1))

        # rate_row[0, t] = relu(inputs[t] @ a), flat on partition 0.
        rate_row = pool.tile([1, K], f32)
        for m in range(0, K, 512):
            n = min(512, K - m)
            r_ps = psp.tile([1, 512], f32, tag="r_ps")
            nc.tensor.matmul(
                r_ps[:, :n], lhsT=a_sb, rhs=xT_sb[:, m : m + n], start=True, stop=True
            )
            nc.scalar.activation(
                rate_row[:, m : m + n], r_ps[:, :n], mybir.ActivationFunctionType.Relu
            )

        # The sequential chain: one fused DVE op per step computing
        # accum = sum_j (wvec_j * rate_t) * s_window_j  (rate_t as the
        # per-partition scalar operand read straight from rate_row).
        # Operand order matters for IEEE fidelity: wvec*rate stays small
        # (never overflows, keeps the exact 0 at tap 0), matching the
        # reference's 0*inf -> NaN and overflow onsets.
        junk = pool.tile([1, WIN], f32)
        for t in range(K):
            nc.vector.scalar_tensor_tensor(
                out=junk,
                in0=w_row,
                scalar=rate_row[:, t : t + 1],
                in1=s_buf[:, t : t + WIN],
                op0=mult,
                op1=mult,
                accum_out=s_buf[:, WIN + t : WIN + t + 1],
            )

        nc.sync.dma_start(
            out=st_out.rearrange("(o k) -> o k", o=1), in_=s_buf[:, K : K + WIN]
        )

        # Observation FIR out[t] = sum_j s[t+j] * diff[j]: 20-tap
        # scalar_tensor_tensor chain over shifted views of s_buf.
        acc = [pool.tile([1, K], f32, name="acc0"), pool.tile([1, K], f32, name="acc1")]
        nc.vector.tensor_scalar_mul(acc[0], s_buf[:, 0:K], d_row[:, 0:1])
        for j in range(1, WIN):
            nc.vector.scalar_tensor_tensor(
                out=acc[j % 2],
                in0=s_buf[:, j : j + K],
                scalar=d_row[:, j : j + 1],
                in1=acc[(j + 1) % 2],
                op0=mult,
                op1=add,
            )
        nc.sync.dma_start(
            out=out[0:K].rearrange("(o k) -> o k", o=1), in_=acc[(WIN - 1) % 2]
        )

        if full_out:
            # NaN tail: one DRAM->DRAM broadcast DMA, no engine time.
            tail_dst = out[K:T_FULL].rearrange("(r c) -> r c", c=NAN_COLS)
            tail_src = bass.AP(
                tensor=nansrc.tensor, offset=nansrc.offset, ap=[[0, 32], [1, NAN_COLS]]
            )
            nc.sync.dma_start(out=tail_dst, in_=tail_src)

    nc.compile()
    return nc


def _get_neff(K, full_out):
    key = (K, full_out)
    if key not in _cache:
        _cache[key] = _build(K, full_out)
    return _cache[key]


def _run(nc, feeds, trace=False):
    from concourse import bass_utils

    in_maps = [dict(feeds) for _ in range(8)]
    res = bass_utils.run_bass_kernel_spmd(nc, in_maps, core_ids=list(range(8)), trace=trace)
    return res


def _feeds(inputs, a, wvec, dvec, state, t0, K, full_out):
    f = {
        "xT": np.ascontiguousarray(inputs[t0 : t0 + K].T),
        "a": a,
        "wv": wvec,
        "dv": dvec,
        "st_in": state,
    }
    if full_out:
        f["nansrc"] = np.full(NAN_COLS, np.nan, dtype=np.float32)
    return f


def kernel(inputs, a, h):
    inputs = np.ascontiguousarray(np.asarray(inputs, dtype=np.float32))
    a = np.ascontiguousarray(np.asarray(a, dtype=np.float32))
    wvec, dvec = _weights_from_h(h)
    state0 = np.zeros(WIN, dtype=np.float32)
    state0[-1] = 1.0

    nc = _get_neff(K_HAPPY, True)
    res = _run(nc, _feeds(inputs, a, wvec, dvec, state0, 0, K_HAPPY, True))
    r0 = res.results[0]
    out = np.array(r0["out"], dtype=np.float32)
    state = np.array(r0["st_out"], dtype=np.float32)

    if np.isnan(state).all():
        # Certified: every later step is NaN irrespective of the rates.
        return out

    # Generic fallback: continue the exact recurrence chunk by chunk.
    nc_c = _get_neff(K_CHUNK, False)
    t = K_HAPPY
    while t < T_FULL:
        k = min(K_CHUNK, T_FULL - t)
        xc = inputs[t : t + k]
        if k < K_CHUNK:  # pad (rates of padded rows can't affect emitted outputs)
            xc = np.concatenate([xc, np.zeros((K_CHUNK - k, NW), np.float32)], axis=0)
        rc = _run(nc_c, _feeds(xc, a, wvec, dvec, state, 0, K_CHUNK, False)).results[0]
        out[t : t + k] = np.array(rc["out"], dtype=np.float32)[:k]
        state = np.array(rc["st_out"], dtype=np.float32)
        t += k
        if np.isnan(state).all() and t < T_FULL:
            out[t:] = np.nan
            break
    return out


# revision 19
# speedup vs baseline: 1.1553x; 1.1553x over previous
"""Trainium2 Bass kernel for nn_Covid19InfectionsPredictModel.

Model: rate = relu(inputs @ a)  [T];  a strictly-sequential 20-tap linear
recurrence s_t = rate_t * dot(s_{t-20..t-1}, rev_head) seeded by a unit
impulse, and the observation FIR out_t = dot(s_{t-20..t-1}, diff).

Key structural fact (pure IEEE-754 float32, no approximation): the weight
rev_head[0] is exactly 0 and diff[j] > 0 for all j. The recurrence explodes
(growth ~14x/step for the given parameter scale), hits +inf, and then
0*inf => NaN poisons the window. Once the 20-value state window is ALL-NaN
at some step t*, every later s_t and out_t is NaN *regardless of rate*
(NaN*w + ... = NaN, and rate*NaN = NaN even for rate == 0). So the exact
full output is determined by the first ~60 steps plus a runtime-verified
all-NaN certificate.

Strategy:
  1. One small NEFF computes the first K=96 steps exactly on-device:
     rate via TensorE matmuls (the host passes the 96-row input slice
     pre-transposed - a pure layout change), per-step pre-scaled weights
     W[t] = rate_t * rev_head as a TensorE outer product flattened
     through a DRAM bounce, the sequential recurrence as ONE fused DVE
     scalar_tensor_tensor (accum_out) per step, the output FIR as a
     20-tap scalar_tensor_tensor chain, plus the NaN tail out[96:] as a
     single DRAM->DRAM broadcast DMA and the final 20-value state window.
  2. Host checks the returned state window. All-NaN (the certified,
     overwhelmingly common case) => done. Otherwise a fallback chunk
     NEFF (1024 steps per launch, same math) is compiled lazily and
     looped over the remaining sequence, so the kernel is exactly
     correct for ANY inputs, just slower in the never-taken branch.

The recurrence is strictly sequential and tiny per step; there is no
batch/scenario dimension in this problem instance, so the 8 cores run the
program SPMD-replicated (core 0's output is gathered) - intra-sequence
sharding has nothing to win (see sharding hint). The sequential step is
latency-bound at one small DVE op (~165ns); step-pairing rewrites were
rejected because they provably shift the inf->NaN onset by one position.
"""

from contextlib import ExitStack

import numpy as np

T_FULL = 65536
NW = 64
L = 21
WIN = 20  # recurrence window (L-1 taps)
K_HAPPY = 96
K_CHUNK = 1024
NAN_COLS = 2045  # (T_FULL - K_HAPPY) == 32 * 2045

_cache = {}


def _weights_from_h(h):
    """rev_head (wvec) and diff (dvec) exactly as the reference computes them."""
    h0 = np.float32(np.asarray(h).reshape(-1)[0])
    t = np.arange(L, dtype=np.float32)
    surv = np.exp(-t / h0).astype(np.float32)
    surv = ((surv - surv[-1]) / (np.float32(1.0) - surv[-1])).astype(np.float32)
    rev = surv[::-1].copy()  # reverse_surv, rev[20] == 1, rev[0] == 0
    wvec = rev[:WIN].copy()  # rev_head, wvec[0] == 0 exactly
    dvec = (rev[1:] - rev[:-1]).copy()  # diff, all > 0
    return wvec, dvec


def _build(K, full_out, dvec):
    """Emit + compile the K-step recurrence NEFF. Returns the Bacc object.

    dvec (the observation FIR taps, derived from the input h) is baked into
    the NEFF as immediate scalars; the cache key includes its bytes.
    """
    import concourse.bacc as bacc
    import concourse.bass as bass
    import concourse.tile as tile
    from concourse import mybir

    f32 = mybir.dt.float32
    mult = mybir.AluOpType.mult
    add = mybir.AluOpType.add

    nc = bacc.Bacc("TRN2", num_devices=1, debug=False)
    # xTa = [inputs[t0:t0+K].T | a] : one DMA covers both matmul operands.
    xTa = nc.dram_tensor("xTa", [NW, K + 1], f32, kind="ExternalInput").ap()
    # aux = [wvec(20) | state0(20)] : one DMA lands wvec and the s-buffer seed.
    aux = nc.dram_tensor("aux", [2 * WIN], f32, kind="ExternalInput").ap()
    if full_out:
        nansrc = nc.dram_tensor("nansrc", [NAN_COLS], f32, kind="ExternalInput").ap()
        out = nc.dram_tensor("out", [T_FULL], f32, kind="ExternalOutput").ap()
    else:
        out = nc.dram_tensor("out", [K], f32, kind="ExternalOutput").ap()
    st_out = nc.dram_tensor("st_out", [WIN], f32, kind="ExternalOutput").ap()

    with tile.TileContext(nc) as tc, ExitStack() as ctx:
        pool = ctx.enter_context(tc.tile_pool(name="p", bufs=1))
        psp = ctx.enter_context(tc.tile_pool(name="ps", bufs=1, space="PSUM"))

        xTa_sb = pool.tile([NW, K + 1], f32)
        nc.sync.dma_start(out=xTa_sb, in_=xTa)
        # aux_sb[0, 0:20] = wvec; aux_sb[0, 20:40] = state0 = s_buf[0:20].
        aux_sb = pool.tile([1, WIN + K + WIN], f32)
        nc.sync.dma_start(out=aux_sb[:, : 2 * WIN], in_=aux.rearrange("(o k) -> o k", o=1))
        w_row = aux_sb[:, 0:WIN]
        s_buf = aux_sb[:, WIN : WIN + K + WIN]

        # rate_row[0, t] = relu(inputs[t] @ a), flat on partition 0 (relu as a
        # DVE max against 0.0 straight out of PSUM - no ScalarE on this path).
        rate_row = pool.tile([1, K], f32)
        for m in range(0, K, 512):
            n = min(512, K - m)
            r_ps = psp.tile([1, 512], f32, tag="r_ps")
            nc.tensor.matmul(
                r_ps[:, :n],
                lhsT=xTa_sb[:, K : K + 1],
                rhs=xTa_sb[:, m : m + n],
                start=True,
                stop=True,
            )
            nc.vector.tensor_scalar_max(rate_row[:, m : m + n], r_ps[:, :n], 0.0)

        # Pre-scaled per-step weights W[t, j] = rate_t * wvec[j]: TensorE outer
        # product, flattened t-major by a single SBUF->SBUF DMA so each step
        # reads a contiguous 20-element slice on partition 0.
        w_ps = psp.tile([K, WIN], f32, tag="w_ps")
        nc.tensor.matmul(w_ps, lhsT=rate_row, rhs=w_row, start=True, stop=True)
        w2d = pool.tile([K, WIN], f32)
        nc.scalar.activation(w2d, w_ps, mybir.ActivationFunctionType.Copy)
        wf = pool.tile([1, K * WIN], f32)
        nc.sync.dma_start(out=wf, in_=w2d)

        # The sequential chain: one fused multiply+accumulate-reduce per step.
        junk = pool.tile([1, WIN], f32)
        for t in range(K):
            o = t * WIN
            nc.vector.scalar_tensor_tensor(
                out=junk,
                in0=s_buf[:, t : t + WIN],
                scalar=1.0,
                in1=wf[:, o : o + WIN],
                op0=mult,
                op1=mult,
                accum_out=s_buf[:, WIN + t : WIN + t + 1],
            )

        nc.gpsimd.dma_start(
            out=st_out.rearrange("(o k) -> o k", o=1), in_=s_buf[:, K : K + WIN]
        )

        # Observation FIR out[t] = sum_j s[t+j] * diff[j]: 20-tap
        # scalar_tensor_tensor chain with the diff taps baked as immediates.
        acc = [pool.tile([1, K], f32, name="acc0"), pool.tile([1, K], f32, name="acc1")]
        nc.vector.tensor_scalar_mul(acc[0], s_buf[:, 0:K], float(dvec[0]))
        for j in range(1, WIN):
            nc.vector.scalar_tensor_tensor(
                out=acc[j % 2],
                in0=s_buf[:, j : j + K],
                scalar=float(dvec[j]),
                in1=acc[(j + 1) % 2],
                op0=mult,
                op1=add,
            )
        nc.sync.dma_start(
            out=out[0:K].rearrange("(o k) -> o k", o=1), in_=acc[(WIN - 1) % 2]
        )

        if full_out:
            # NaN tail: one DRAM->DRAM broadcast DMA on the GPSIMD queues,
            # no engine time and no Sync-sequencer issue slot.
            tail_dst = out[K:T_FULL].rearrange("(r c) -> r c", c=NAN_COLS)
            tail_src = bass.AP(
                tensor=nansrc.tensor, offset=nansrc.offset, ap=[[0, 32], [1, NAN_COLS]]
            )
            nc.gpsimd.dma_start(out=tail_dst, in_=tail_src)

    nc.compile()
    return nc


def _get_neff(K, full_out, dvec):
    key = (K, full_out, dvec.tobytes())
    if key not in _cache:
        _cache[key] = _build(K, full_out, dvec)
    return _cache[key]


def _run(nc, feeds, trace=False):
    from concourse import bass_utils

    in_maps = [dict(feeds) for _ in range(8)]
    res = bass_utils.run_bass_kernel_spmd(nc, in_maps, core_ids=list(range(8)), trace=trace)
    return res


def _feeds(inputs, a, wvec, state, t0, K, full_out):
    xTa = np.empty((NW, K + 1), dtype=np.float32)
    xTa[:, :K] = inputs[t0 : t0 + K].T
    xTa[:, K] = a
    aux = np.concatenate([wvec, state]).astype(np.float32)
    f = {"xTa": xTa, "aux": aux}
    if full_out:
        f["nansrc"] = np.full(NAN_COLS, np.nan, dtype=np.float32)
    return f


def kernel(inputs, a, h):
    inputs = np.ascontiguousarray(np.asarray(inputs, dtype=np.float32))
    a = np.ascontiguousarray(np.asarray(a, dtype=np.float32))
    wvec, dvec = _weights_from_h(h)
    state0 = np.zeros(WIN, dtype=np.float32)
    state0[-1] = 1.0

    nc = _get_neff(K_HAPPY, True, dvec)
    res = _run(nc, _feeds(inputs, a, wvec, state0, 0, K_HAPPY, True))
    r0 = res.results[0]
    out = np.array(r0["out"], dtype=np.float32)
    state = np.array(r0["st_out"], dtype=np.float32)

    if np.isnan(state).all():
        # Certified: every later step is NaN irrespective of the rates.
        return out

    # Generic fallback: continue the exact recurrence chunk by chunk.
    nc_c = _get_neff(K_CHUNK, False, dvec)
    t = K_HAPPY
    while t < T_FULL:
        k = min(K_CHUNK, T_FULL - t)
        xc = inputs[t : t + k]
        if k < K_CHUNK:  # pad (rates of padded rows can't affect emitted outputs)
            xc = np.concatenate([xc, np.zeros((K_CHUNK - k, NW), np.float32)], axis=0)
        rc = _run(nc_c, _feeds(xc, a, wvec, state, 0, K_CHUNK, False)).results[0]
        out[t : t + k] = np.array(rc["out"], dtype=np.float32)[:k]
        state = np.array(rc["st_out"], dtype=np.float32)
        t += k
        if np.isnan(state).all() and t < T_FULL:
            out[t:] = np.nan
            break
    return out
